# revision 22
# baseline (speedup 1.0000x reference)
"""AttentionBlock kernel for Trainium2, data-parallel over batch on 8 NeuronCores.

Per-core computation (one batch element, x_b: [256, 4096] = [C, H*W]):
  GroupNorm(8 groups) folded into the QKV projection:
    xn = x*scale_c + shift_c   (per-channel affine from group stats)
    qkv = W_qkv xn + b  ==  (W_qkv * scale_c) x + (W_qkv shift + b)
  All heavy matmuls run in fp8 with the DoubleRow perf mode (2 contraction
  elements per partition per cycle => K=256 contractions take one 0.5-cyc/row
  matmul instead of two 1-cyc/row fp32r matmuls):
    x8    [128, 2, N]  e4m3  (dim1 = input-channel tile)
    wadj8 [128, 2, 3C] e4m3  (GN scale folded; dim1 = input-channel tile)
    q,k   -> qpack/kpack [128, 2, N] e4m3 (dim1 = output-channel tile)
    v     -> vT [128, 16, 2, C] e4m3 (m on partitions; dims = pair, parity)
  Attention: S'[m,n] = sum_c k[c,m] q[c,n] (m on partitions), one DoubleRow
  matmul per (nb, mb).  P' = exp(S'/16) quantized straight to fp8 e5m2:
    - ACT engine tiles: native Exp activation with fp8e5 output
    - DVE engine tiles: Schraudolph trick - i8 = round(s*log2e/4/16 + 60.25)
      and the int8 BITS are the e5m2 value of exp(s/16) (max rel err ~13%,
      fine: softmax-averaged impact ~3e-3 absolute on the output).
  out[c,n] accumulates in PSUM over 16 DoubleRow matmuls (mb pairs), the
  denominator d[n] rides a ones-column DoubleRow matmul into one PSUM row.
  v-bias is folded out of the hot loop: out_norm = P v0 / d + bv, so
  proj(out_norm) = proj(P v0 / d) + (wpT bv), precomputed into the proj bias.
  Normalization: 1/d computed on the [1,512] row (DVE cost only depends on
  free size), broadcast via a K=1 matmul, consumed directly from PSUM:
    y = (proj(out_unnorm) * rdb) + (proj_b + wpT bv) + x
"""

import sys

sys.path.insert(0, "/opt/trn_rl_repo")

import numpy as np

import concourse.bass as bass  # noqa: F401
import concourse.mybir as mybir
import concourse.tile as tile
from concourse import bacc
from concourse.bass_utils import run_bass_kernel_spmd

F32 = mybir.dt.float32
F32R = mybir.dt.float32r
F8E4 = mybir.dt.float8e4
F8E5 = mybir.dt.float8e5
I8 = mybir.dt.int8
AF = mybir.ActivationFunctionType
ALU = mybir.AluOpType
DR = mybir.MatmulPerfMode.DoubleRow

C = 256
N = 4096
GROUPS = 8
EPS = 1e-5
CT = 2          # channel tiles of 128
MT = 32         # m (key/token) tiles of 128
PAIRS = MT // 2
NB = 8          # n (query/token) chunks of 512
NCHUNK = 512
SCALE = 1.0 / 16.0  # 1/sqrt(C)
GSIZE = C // GROUPS
GN_COUNT = float(GSIZE * N)
XCH = 4         # x DMA/stat chunks per c-tile
XCW = N // XCH  # 2048

# Schraudolph exp -> e5m2 bits: value(bits i) ~= 2^(i/4 - 15); we want
# e^(s/16) = 2^(s*log2e/16), so i = s * (4*log2e/16) + 60 (-0.24: the DVE
# fp32->int8 convert rounds to nearest (HW-probed), and the mantissa
# linearization wants a small negative shift to center the log error).
SCH_A = float(np.log2(np.e) / 4.0)
SCH_B = 59.76
# pairs evicted on DVE; the rest on ACT (10/6 pair split per nb)
DVE_PAIRS = (2, 5, 7, 10, 12, 15)


def _build():
    nc = bacc.Bacc("TRN2", target_bir_lowering=False)

    x_d = nc.declare_dram_parameter("x", [C, N], F32R, isOutput=False)
    wqkvT_d = nc.declare_dram_parameter("wqkvT", [C, 3 * C], F32, isOutput=False)
    wpT_d = nc.declare_dram_parameter("wpT", [C, C], F32R, isOutput=False)
    bqk_d = nc.declare_dram_parameter("bqk", [128, 4], F32, isOutput=False)
    bv_d = nc.declare_dram_parameter("bv", [128, 2], F32, isOutput=False)
    bp_d = nc.declare_dram_parameter("bp", [128, 2], F32, isOutput=False)
    gamma_d = nc.declare_dram_parameter("gamma", [128, 2], F32, isOutput=False)
    beta_d = nc.declare_dram_parameter("beta", [128, 2], F32, isOutput=False)
    sel_d = nc.declare_dram_parameter("sel", [128, 2 * GROUPS], F32, isOutput=False)
    selb_d = nc.declare_dram_parameter("selb", [GROUPS, C], F32, isOutput=False)
    ones_d = nc.declare_dram_parameter("ones", [128, 128], F32R, isOutput=False)
    out_d = nc.declare_dram_parameter("out", [C, N], F32, isOutput=True)

    with tile.TileContext(nc) as tc:
        with (
            nc.allow_low_precision(reason="fp8 attention by design; error budget 2e-2"),
            tc.tile_pool(name="const", bufs=1) as cp,
            tc.tile_pool(name="work", bufs=1) as wp,
        ):
            ones = cp.tile([128, 128], F32R, name="ones", tag="ones")
            nc.sync.dma_start(ones[:], ones_d[:])
            ones8 = cp.tile([128, 2, 128], F8E4, name="ones8", tag="ones8")
            nc.any.memset(ones8[:], 1.0)
            # ---- x loads (chunked so GN stats + fp8 casts overlap the DMA) ----
            xt = []
            for t in range(CT):
                xtile = cp.tile([128, N], F32R, name=f"x{t}", tag=f"x{t}")
                for ch in range(XCH):
                    nc.sync.dma_start(xtile[:, ch * XCW:(ch + 1) * XCW],
                                      x_d[t * 128:(t + 1) * 128, ch * XCW:(ch + 1) * XCW])
                xt.append(xtile)
            # ---- GN statistics per chunk: sx partials (DVE) | sxx partials (ACT) ----
            stats = []
            x8 = cp.tile([128, CT, N], F8E4, name="x8", tag="x8")
            for t in range(CT):
                st = cp.tile([128, 2 * XCH], F32, name=f"stats{t}", tag=f"stats{t}")
                for ch in range(XCH):
                    xv = xt[t][:, ch * XCW:(ch + 1) * XCW].bitcast(F32)
                    nc.vector.tensor_reduce(st[:, ch:ch + 1], xv, mybir.AxisListType.X, ALU.add)
                    scratch = wp.tile([128, XCW], F32, tag="scratch", name="scratch")
                    nc.scalar.activation(scratch[:], xv, AF.Square,
                                         accum_out=st[:, XCH + ch:XCH + ch + 1])
                    # fp8 cast of the same chunk (split across engines)
                    x8dst = x8[:, t, ch * XCW:(ch + 1) * XCW]
                    if (t * XCH + ch) % 2 == 0:
                        nc.vector.tensor_copy(x8dst, xv)
                    else:
                        nc.scalar.copy(x8dst, xv)
                stats.append(st)

            # ---- remaining loads ----
            wT = []
            wpt = []
            for t in range(CT):
                wtile = cp.tile([128, 3 * C], F32, name=f"wT{t}", tag=f"wT{t}")
                nc.sync.dma_start(wtile[:], wqkvT_d[t * 128:(t + 1) * 128, :])
                wT.append(wtile)
                wptile = cp.tile([128, C], F32R, name=f"wpT{t}", tag=f"wpT{t}")
                nc.sync.dma_start(wptile[:], wpT_d[t * 128:(t + 1) * 128, :])
                wpt.append(wptile)
            bqk = cp.tile([128, 4], F32, name="bqk", tag="bqk")
            nc.sync.dma_start(bqk[:], bqk_d[:])
            bv = cp.tile([128, 2], F32, name="bv", tag="bv")
            nc.sync.dma_start(bv[:], bv_d[:])
            bp = cp.tile([128, 2], F32, name="bp", tag="bp")
            nc.sync.dma_start(bp[:], bp_d[:])
            gamma = cp.tile([128, 2], F32, name="gamma", tag="gamma")
            nc.sync.dma_start(gamma[:], gamma_d[:])
            beta = cp.tile([128, 2], F32, name="beta", tag="beta")
            nc.sync.dma_start(beta[:], beta_d[:])
            sel = cp.tile([128, 2 * GROUPS], F32, name="sel", tag="sel")
            nc.sync.dma_start(sel[:], sel_d[:])
            selb = cp.tile([GROUPS, C], F32, name="selb", tag="selb")
            nc.sync.dma_start(selb[:], selb_d[:])

            # ---- setup-phase PSUM pool (closed before the attention loop) ----
            with tc.tile_pool(name="ps0", bufs=2, space="PSUM") as ps0:
                # dummy matmuls keep the PE HAM-warm while GN stats run
                for wi in range(12):
                    wps = ps0.tile([128, NCHUNK], F32, tag="warm", name="wps")
                    wch = min(wi * 2 * XCH // 12, 2 * XCH - 1)  # chunk index 0..7
                    cw = N // XCH
                    base = (wch % XCH) * cw
                    nc.tensor.matmul(wps[:], ones[:],
                                     xt[wch // XCH][:, base:base + NCHUNK],
                                     start=True, stop=True)
                g_ps = ps0.tile([GROUPS, 2 * XCH], F32, tag="small", name="g_ps")
                nc.tensor.matmul(g_ps[:], sel[:, 0:GROUPS], stats[0][:], start=True, stop=False)
                nc.tensor.matmul(g_ps[:], sel[:, GROUPS:2 * GROUPS], stats[1][:], start=False, stop=True)
                # per-group mean / rstd on partitions 0..7
                g_mr = cp.tile([GROUPS, 2], F32, name="g_mr", tag="g_mr")
                gtmp = cp.tile([GROUPS, 5], F32, name="gtmp", tag="gtmp")
                g_sb = cp.tile([GROUPS, 2 * XCH], F32, name="g_sb", tag="g_sb")
                nc.scalar.copy(g_sb[:], g_ps[:])
                nc.vector.tensor_reduce(gtmp[:, 3:4], g_sb[:, 0:XCH],
                                        mybir.AxisListType.X, ALU.add)
                nc.vector.tensor_reduce(gtmp[:, 4:5], g_sb[:, XCH:2 * XCH],
                                        mybir.AxisListType.X, ALU.add)
                nc.vector.tensor_scalar_mul(g_mr[:, 0:1], gtmp[:, 3:4], 1.0 / GN_COUNT)
                nc.vector.tensor_scalar_mul(gtmp[:, 0:1], gtmp[:, 4:5], 1.0 / GN_COUNT)
                nc.vector.tensor_mul(gtmp[:, 1:2], g_mr[:, 0:1], g_mr[:, 0:1])
                nc.vector.tensor_sub(gtmp[:, 2:3], gtmp[:, 0:1], gtmp[:, 1:2])
                gvar = cp.tile([GROUPS, 1], F32, name="gvar", tag="gvar")
                nc.vector.tensor_scalar_add(gvar[:], gtmp[:, 2:3], EPS)
                gstd = cp.tile([GROUPS, 1], F32, name="gstd", tag="gstd")
                nc.scalar.activation(gstd[:], gvar[:], AF.Sqrt)
                nc.vector.reciprocal(g_mr[:, 1:2], gstd[:])

                # broadcast group mean/rstd to per-channel scale/shift
                scale_t = []
                shift_t = []
                for t in range(CT):
                    mr_ps = ps0.tile([128, 2], F32, tag="small", name="mr_ps")
                    nc.tensor.matmul(mr_ps[:], selb[:, t * 128:(t + 1) * 128], g_mr[:],
                                     start=True, stop=True)
                    mr = cp.tile([128, 2], F32, name=f"mr{t}", tag=f"mr{t}")
                    nc.scalar.copy(mr[:], mr_ps[:])
                    sc = cp.tile([128, 1], F32, name=f"scale{t}", tag=f"scale{t}")
                    nc.vector.tensor_mul(sc[:], mr[:, 1:2], gamma[:, t:t + 1])
                    tmp = cp.tile([128, 1], F32, name=f"mscale{t}", tag=f"mscale{t}")
                    nc.vector.tensor_mul(tmp[:], mr[:, 0:1], sc[:])
                    sh = cp.tile([128, 1], F32, name=f"shift{t}", tag=f"shift{t}")
                    nc.vector.tensor_sub(sh[:], beta[:, t:t + 1], tmp[:])
                    scale_t.append(sc)
                    shift_t.append(sh)

                # adjusted qkv weights in fp8: wadj8[c, t, o] = wT[c, o] * scale_c
                wadj8 = cp.tile([128, CT, 3 * C], F8E4, name="wadj8", tag="wadj8")
                for t in range(CT):
                    nc.vector.tensor_scalar_mul(wadj8[:, t, :], wT[t][:], scale_t[t][:])
                # q/k bias: btot[o] = qkv_b[o] + sum_c wT[c,o]*shift_c  (o in 0..512)
                bias_ps = ps0.tile([128, 4], F32, tag="small", name="bias_ps")
                for ot in range(4):
                    for t in range(CT):
                        nc.tensor.matmul(bias_ps[:, ot:ot + 1],
                                         wT[t][:, ot * 128:(ot + 1) * 128],
                                         shift_t[t][:],
                                         start=(t == 0), stop=(t == CT - 1))
                btot = cp.tile([128, 4], F32, name="btot", tag="btot")
                nc.vector.tensor_add(btot[:], bias_ps[:], bqk[:])
                # v bias columns: bvcol[c, t] = qkv_b_v[c] + sum_i shift_i wvT[i, c]
                bv_ps = ps0.tile([128, 2], F32, tag="small", name="bv_ps")
                for t in range(CT):
                    for t2 in range(CT):
                        nc.tensor.matmul(bv_ps[:, t:t + 1],
                                         wT[t2][:, 2 * C + t * 128:2 * C + (t + 1) * 128],
                                         shift_t[t2][:],
                                         start=(t2 == 0), stop=(t2 == CT - 1))
                bvcol = cp.tile([128, 2], F32, name="bvcol", tag="bvcol")
                nc.vector.tensor_add(bvcol[:], bv_ps[:], bv[:])
                # fold v bias through proj: pbv[o] = sum_c wpT[c, o] * bvcol[c]
                pbv_ps = ps0.tile([128, 2], F32, tag="small", name="pbv_ps")
                for ot in range(CT):
                    for t in range(CT):
                        nc.tensor.matmul(pbv_ps[:, ot:ot + 1],
                                         wpt[t][:, ot * 128:(ot + 1) * 128].bitcast(F32),
                                         bvcol[:, t:t + 1],
                                         start=(t == 0), stop=(t == CT - 1))
                bp_tot = cp.tile([128, 2], F32, name="bp_tot", tag="bp_tot")
                nc.vector.tensor_add(bp_tot[:], pbv_ps[:], bp[:])

            with tc.tile_pool(name="ps1", bufs=1, space="PSUM") as ps1:
                # ---- q/k projections (DoubleRow): 2 n-chunks per PSUM tile ----
                qpack = cp.tile([128, 2, N], F8E4, name="qpack", tag="qpack")
                kpack = cp.tile([128, 2, N], F8E4, name="kpack", tag="kpack")
                # k fully first (attention needs all of k); then q in n-major
                # order so the first attention chunk unblocks after 2 evictions
                order = ([(2, 0), (3, 0), (0, 0), (1, 0)] +
                         [(ot, mcp) for mcp in range(1, NB // 2) for ot in (2, 3)] +
                         [(ot, mcp) for mcp in range(1, NB // 2) for ot in (0, 1)])
                for ev, (ot, mcp) in enumerate(order):
                    dst_t, tq = (qpack, ot) if ot < 2 else (kpack, ot - 2)
                    qk_ps = ps1.tile([128, 2 * NCHUNK], F32, tag="qk", bufs=3, name="qk_ps")
                    for half in range(2):
                        mc = 2 * mcp + half
                        nc.tensor.matmul(qk_ps[:, half * NCHUNK:(half + 1) * NCHUNK],
                                         wadj8[:, 0:2, ot * 128:(ot + 1) * 128],
                                         x8[:, 0:2, mc * NCHUNK:(mc + 1) * NCHUNK],
                                         start=True, stop=True, perf_mode=DR)
                    dst = dst_t[:, tq, 2 * mcp * NCHUNK:(2 * mcp + 2) * NCHUNK]
                    if ev % 2 == 0:
                        nc.scalar.activation(dst, qk_ps[:], AF.Identity,
                                             bias=btot[:, ot:ot + 1])
                    else:
                        nc.vector.tensor_scalar_add(dst, qk_ps[:], btot[:, ot:ot + 1])

            # vT[m, pair, parity, c] produced inside the first nb iteration so
            # its evictions pipeline with the attention start.
            vT = cp.tile([128, PAIRS, 2, C], F8E4, name="vT", tag="vT")

            with tc.tile_pool(name="ps", bufs=1, space="PSUM") as ps:
                # ---- attention ----
                # The proj/normalize epilogue of chunk nb-1 is deferred into
                # the start of chunk nb (after pair 1) so the PE never stalls
                # on the att evictions: the next chunk's S matmuls fill the
                # wait.  att evict + reciprocal stay at the end of their own
                # chunk (ring-reuse safety: readers issue before reuse).
                def emit_z(att_p, ot):
                    z_ps = ps.tile([128, NCHUNK], F32, tag="z", bufs=1, name="z_ps")
                    for t in range(CT):
                        nc.tensor.matmul(z_ps[:],
                                         wpt[t][:, ot * 128:(ot + 1) * 128],
                                         att_p[t][:],
                                         start=(t == 0), stop=(t == CT - 1))
                    return z_ps

                def emit_y(z_ps, nsl_p, rdb_p, ot):
                    y = wp.tile([128, NCHUNK], F32, tag="y", bufs=4, name="y")
                    nc.vector.tensor_mul(y[:], z_ps[:], rdb_p[:])
                    nc.vector.scalar_tensor_tensor(
                        y[:], in0=y[:], scalar=bp_tot[:, ot:ot + 1],
                        in1=xt[ot][:, nsl_p].bitcast(F32), op0=ALU.add, op1=ALU.add)
                    nc.sync.dma_start(out_d[ot * 128:(ot + 1) * 128, nsl_p], y[:])

                pending = None
                for nb in range(NB):
                    nsl = slice(nb * NCHUNK, (nb + 1) * NCHUNK)
                    out_ps = [ps.tile([128, NCHUNK], F32, tag="out", bufs=2, name=f"outp{_t}")
                              for _t in range(CT)]
                    d_ps = ps.tile([128, NCHUNK], F32, tag="d", bufs=1, name="d_ps")
                    def emit_out_d(pair, p2p):
                        first, last = (pair == 0), (pair == PAIRS - 1)
                        for t in range(CT):
                            nc.tensor.matmul(out_ps[t][:],
                                             vT[:, pair, 0:2, t * 128:(t + 1) * 128],
                                             p2p[:, 0:2, :],
                                             start=first, stop=last, perf_mode=DR)
                        nc.tensor.matmul(d_ps[:], ones8[:, 0:2, 0:128], p2p[:, 0:2, :],
                                         start=first, stop=last, perf_mode=DR)

                    # mb loop software-pipelined by one pair: pair p's S+exp
                    # issue before pair p-1's out/d matmuls, so a late exp
                    # never blocks ready S work in the PE FIFO.
                    prev_p2 = None
                    for pair in range(PAIRS):
                        if nb == 0:
                            # produce vT pair: two DoubleRow matmuls + one evict
                            vt_ps = ps.tile([128, 2 * C], F32, tag="z", bufs=1, name="vt_ps")
                            for par in range(2):
                                mt = 2 * pair + par
                                nc.tensor.matmul(vt_ps[:, par * C:(par + 1) * C],
                                                 x8[:, 0:2, mt * 128:(mt + 1) * 128],
                                                 wadj8[:, 0:2, 2 * C:3 * C],
                                                 start=True, stop=True, perf_mode=DR)
                            vdst = vT[:, pair, 0:2, :]
                            if pair % 2 == 0:
                                nc.scalar.copy(vdst, vt_ps[:])
                            else:
                                nc.vector.tensor_copy(vdst, vt_ps[:])
                        p2 = wp.tile([128, 2, NCHUNK], F8E5, tag="p", bufs=6, name="p2")
                        s2 = ps.tile([128, 2, NCHUNK], F32, tag="s", bufs=2, name="s2")
                        for par in range(2):
                            mb = 2 * pair + par
                            nc.tensor.matmul(s2[:, par, :],
                                             kpack[:, 0:2, mb * 128:(mb + 1) * 128],
                                             qpack[:, 0:2, nsl],
                                             start=True, stop=True, perf_mode=DR)
                        # one [128,1024] eviction covers the whole pair
                        if pair in DVE_PAIRS:
                            nc.vector.tensor_scalar(
                                p2[:, 0:2, :].bitcast(I8), s2[:, 0:2, :],
                                SCH_A, SCH_B, op0=ALU.mult, op1=ALU.add)
                        else:
                            nc.scalar.activation(p2[:, 0:2, :], s2[:, 0:2, :],
                                                 AF.Exp, scale=SCALE)
                        if pending is not None:
                            if pair == 0:
                                z_prev = emit_z(pending[1], 0)
                            elif pair == 1:
                                emit_y(z_prev, pending[0], pending[2], 0)
                            elif pair == 2:
                                z_prev = emit_z(pending[1], 1)
                            elif pair == 3:
                                emit_y(z_prev, pending[0], pending[2], 1)
                                pending = None
                        if prev_p2 is not None:
                            emit_out_d(pair - 1, prev_p2)
                        prev_p2 = p2
                    emit_out_d(PAIRS - 1, prev_p2)
                    # d_ps holds the denominator broadcast on all 128
                    # partitions (ones weights are 128 wide): approx
                    # reciprocal (18 bits, plenty) straight to SBUF
                    rdb = wp.tile([128, NCHUNK], F32, tag="rdb", bufs=2, name="rdb")
                    nc.vector.reciprocal_approx_fast(rdb[:], d_ps[:])
                    # evict attention numerator (unnormalized) to SBUF
                    att = []
                    for t in range(CT):
                        at = wp.tile([128, NCHUNK], F32R, tag="att", bufs=4, name="att")
                        if t == 0:
                            nc.scalar.copy(at[:], out_ps[t][:])
                        else:
                            nc.vector.tensor_copy(at[:], out_ps[t][:])
                        att.append(at)
                    pending = (nsl, att, rdb)
                for ot in range(CT):
                    zp = emit_z(pending[1], ot)
                    emit_y(zp, pending[0], pending[2], ot)
    nc.compile()
    return nc


_NC = None


def _get_nc():
    global _NC
    if _NC is None:
        _NC = _build()
    return _NC


def _prepare_in_maps(x, gn_w, gn_b, qkv_w, qkv_b, proj_w, proj_b):
    x = np.asarray(x, dtype=np.float32)
    b = x.shape[0]
    assert b == 8 and x.shape[1] == C
    xs = x.reshape(b, C, N)

    wqkvT = np.ascontiguousarray(np.asarray(qkv_w, np.float32).T)      # [C, 3C]
    wpT = np.ascontiguousarray(np.asarray(proj_w, np.float32).T)       # [C, C]
    qkv_b = np.asarray(qkv_b, np.float32)
    bqk = np.ascontiguousarray(qkv_b[:2 * C].reshape(4, 128).T)        # [128, 4]
    bv = np.ascontiguousarray(qkv_b[2 * C:].reshape(CT, 128).T)        # [128, 2]
    bp = np.ascontiguousarray(np.asarray(proj_b, np.float32).reshape(CT, 128).T)
    gamma = np.ascontiguousarray(np.asarray(gn_w, np.float32).reshape(CT, 128).T)
    beta = np.ascontiguousarray(np.asarray(gn_b, np.float32).reshape(CT, 128).T)

    # group selectors: channel c -> group c // GSIZE
    sel = np.zeros((128, 2 * GROUPS), np.float32)
    selb = np.zeros((GROUPS, C), np.float32)
    for t in range(CT):
        for p in range(128):
            g = (t * 128 + p) // GSIZE
            sel[p, t * GROUPS + g] = 1.0
            selb[g, t * 128 + p] = 1.0

    shared = {
        "wqkvT": wqkvT, "wpT": wpT, "bqk": bqk, "bv": bv, "bp": bp,
        "gamma": gamma, "beta": beta, "sel": sel, "selb": selb,
        "ones": np.ones((128, 128), np.float32),
    }
    return [dict(shared, x=np.ascontiguousarray(xs[i])) for i in range(b)]


def kernel(x, gn_w, gn_b, qkv_w, qkv_b, proj_w, proj_b):
    x = np.asarray(x, dtype=np.float32)
    in_maps = _prepare_in_maps(x, gn_w, gn_b, qkv_w, qkv_b, proj_w, proj_b)
    nc = _get_nc()
    res = run_bass_kernel_spmd(nc, in_maps, core_ids=list(range(8)))
    out = np.stack([res.results[i]["out"] for i in range(8)])
    return out.reshape(x.shape).astype(np.float32)


# revision 24
# speedup vs baseline: 1.0266x; 1.0266x over previous
"""AttentionBlock kernel for Trainium2, data-parallel over batch on 8 NeuronCores.

Per-core computation (one batch element, x_b: [256, 4096] = [C, H*W]):
  GroupNorm(8 groups) folded into the QKV projection:
    xn = x*scale_c + shift_c   (per-channel affine from group stats)
    qkv = W_qkv xn + b  ==  (W_qkv * scale_c) x + (W_qkv shift + b)
  All heavy matmuls run in fp8 with the DoubleRow perf mode (2 contraction
  elements per partition per cycle => K=256 contractions take one 0.5-cyc/row
  matmul instead of two 1-cyc/row fp32r matmuls):
    x8    [128, 2, N]  e4m3  (dim1 = input-channel tile)
    wadj8 [128, 2, 3C] e4m3  (GN scale folded; dim1 = input-channel tile)
    q,k   -> qpack/kpack [128, 2, N] e4m3 (dim1 = output-channel tile)
    v     -> vT [128, 16, 2, C] e4m3 (m on partitions; dims = pair, parity)
  Attention: S'[m,n] = sum_c k[c,m] q[c,n] (m on partitions), one DoubleRow
  matmul per (nb, mb).  P' = exp(S'/16) quantized straight to fp8 e5m2:
    - ACT engine tiles: native Exp activation with fp8e5 output
    - DVE engine tiles: Schraudolph trick - i8 = round(s*log2e/4/16 + 60.25)
      and the int8 BITS are the e5m2 value of exp(s/16) (max rel err ~13%,
      fine: softmax-averaged impact ~3e-3 absolute on the output).
  out[c,n] accumulates in PSUM over 16 DoubleRow matmuls (mb pairs), the
  denominator d[n] rides a ones-column DoubleRow matmul into one PSUM row.
  v-bias is folded out of the hot loop: out_norm = P v0 / d + bv, so
  proj(out_norm) = proj(P v0 / d) + (wpT bv), precomputed into the proj bias.
  Normalization: 1/d computed on the [1,512] row (DVE cost only depends on
  free size), broadcast via a K=1 matmul, consumed directly from PSUM:
    y = (proj(out_unnorm) * rdb) + (proj_b + wpT bv) + x
"""

import sys

sys.path.insert(0, "/opt/trn_rl_repo")

import numpy as np

import concourse.bass as bass  # noqa: F401
import concourse.mybir as mybir
import concourse.tile as tile
from concourse import bacc
from concourse.bass_utils import run_bass_kernel_spmd

F32 = mybir.dt.float32
F32R = mybir.dt.float32r
F8E4 = mybir.dt.float8e4
F8E5 = mybir.dt.float8e5
I8 = mybir.dt.int8
AF = mybir.ActivationFunctionType
ALU = mybir.AluOpType
DR = mybir.MatmulPerfMode.DoubleRow

C = 256
N = 4096
GROUPS = 8
EPS = 1e-5
CT = 2          # channel tiles of 128
MT = 32         # m (key/token) tiles of 128
PAIRS = MT // 2
NB = 8          # n (query/token) chunks of 512
NCHUNK = 512
SCALE = 1.0 / 16.0  # 1/sqrt(C)
GSIZE = C // GROUPS
GN_COUNT = float(GSIZE * N)
XCH = 4         # x DMA/stat chunks per c-tile
XCW = N // XCH  # 2048

# Schraudolph exp -> e5m2 bits: value(bits i) ~= 2^(i/4 - 15); we want
# e^(s/16) = 2^(s*log2e/16), so i = s * (4*log2e/16) + 60 (-0.24: the DVE
# fp32->int8 convert rounds to nearest (HW-probed), and the mantissa
# linearization wants a small negative shift to center the log error).
SCH_A = float(np.log2(np.e) / 4.0)
SCH_B = 59.76
# mb (mod 16) slots evicted on DVE; the rest on ACT (18/14 split per nb)
DVE_MB = (1, 3, 5, 8, 10, 12, 14)


def _build():
    nc = bacc.Bacc("TRN2", target_bir_lowering=False)

    x_d = nc.declare_dram_parameter("x", [C, N], F32R, isOutput=False)
    wqkvT_d = nc.declare_dram_parameter("wqkvT", [C, 3 * C], F32, isOutput=False)
    wpT_d = nc.declare_dram_parameter("wpT", [C, C], F32R, isOutput=False)
    bqk_d = nc.declare_dram_parameter("bqk", [128, 4], F32, isOutput=False)
    bv_d = nc.declare_dram_parameter("bv", [128, 2], F32, isOutput=False)
    bp_d = nc.declare_dram_parameter("bp", [128, 2], F32, isOutput=False)
    gamma_d = nc.declare_dram_parameter("gamma", [128, 2], F32, isOutput=False)
    beta_d = nc.declare_dram_parameter("beta", [128, 2], F32, isOutput=False)
    sel_d = nc.declare_dram_parameter("sel", [128, 2 * GROUPS], F32, isOutput=False)
    selb_d = nc.declare_dram_parameter("selb", [GROUPS, C], F32, isOutput=False)
    ones_d = nc.declare_dram_parameter("ones", [128, 128], F32R, isOutput=False)
    out_d = nc.declare_dram_parameter("out", [C, N], F32, isOutput=True)

    with tile.TileContext(nc) as tc:
        with (
            nc.allow_low_precision(reason="fp8 attention by design; error budget 2e-2"),
            tc.tile_pool(name="const", bufs=1) as cp,
            tc.tile_pool(name="work", bufs=1) as wp,
        ):
            ones = cp.tile([128, 128], F32R, name="ones", tag="ones")
            nc.sync.dma_start(ones[:], ones_d[:])
            ones8 = cp.tile([128, 2, 128], F8E4, name="ones8", tag="ones8")
            nc.any.memset(ones8[:], 1.0)
            # ---- x loads (chunked so GN stats + fp8 casts overlap the DMA) ----
            xt = []
            for t in range(CT):
                xtile = cp.tile([128, N], F32R, name=f"x{t}", tag=f"x{t}")
                for ch in range(XCH):
                    nc.sync.dma_start(xtile[:, ch * XCW:(ch + 1) * XCW],
                                      x_d[t * 128:(t + 1) * 128, ch * XCW:(ch + 1) * XCW])
                xt.append(xtile)
            # ---- GN statistics per chunk: sx partials (DVE) | sxx partials (ACT) ----
            stats = []
            x8 = cp.tile([128, CT, N], F8E4, name="x8", tag="x8")
            for t in range(CT):
                st = cp.tile([128, 2 * XCH], F32, name=f"stats{t}", tag=f"stats{t}")
                for ch in range(XCH):
                    xv = xt[t][:, ch * XCW:(ch + 1) * XCW].bitcast(F32)
                    nc.vector.tensor_reduce(st[:, ch:ch + 1], xv, mybir.AxisListType.X, ALU.add)
                    scratch = wp.tile([128, XCW], F32, tag="scratch", name="scratch")
                    nc.scalar.activation(scratch[:], xv, AF.Square,
                                         accum_out=st[:, XCH + ch:XCH + ch + 1])
                    # fp8 cast of the same chunk (split across engines)
                    x8dst = x8[:, t, ch * XCW:(ch + 1) * XCW]
                    if (t * XCH + ch) % 2 == 0:
                        nc.vector.tensor_copy(x8dst, xv)
                    else:
                        nc.scalar.copy(x8dst, xv)
                stats.append(st)

            # ---- remaining loads ----
            wT = []
            wpt = []
            for t in range(CT):
                wtile = cp.tile([128, 3 * C], F32, name=f"wT{t}", tag=f"wT{t}")
                nc.sync.dma_start(wtile[:], wqkvT_d[t * 128:(t + 1) * 128, :])
                wT.append(wtile)
                wptile = cp.tile([128, C], F32R, name=f"wpT{t}", tag=f"wpT{t}")
                nc.sync.dma_start(wptile[:], wpT_d[t * 128:(t + 1) * 128, :])
                wpt.append(wptile)
            bqk = cp.tile([128, 4], F32, name="bqk", tag="bqk")
            nc.sync.dma_start(bqk[:], bqk_d[:])
            bv = cp.tile([128, 2], F32, name="bv", tag="bv")
            nc.sync.dma_start(bv[:], bv_d[:])
            bp = cp.tile([128, 2], F32, name="bp", tag="bp")
            nc.sync.dma_start(bp[:], bp_d[:])
            gamma = cp.tile([128, 2], F32, name="gamma", tag="gamma")
            nc.sync.dma_start(gamma[:], gamma_d[:])
            beta = cp.tile([128, 2], F32, name="beta", tag="beta")
            nc.sync.dma_start(beta[:], beta_d[:])
            sel = cp.tile([128, 2 * GROUPS], F32, name="sel", tag="sel")
            nc.sync.dma_start(sel[:], sel_d[:])
            selb = cp.tile([GROUPS, C], F32, name="selb", tag="selb")
            nc.sync.dma_start(selb[:], selb_d[:])

            # ---- setup-phase PSUM pool (closed before the attention loop) ----
            with tc.tile_pool(name="ps0", bufs=2, space="PSUM") as ps0:
                # dummy matmuls keep the PE HAM-warm while GN stats run
                for wi in range(12):
                    wps = ps0.tile([128, NCHUNK], F32, tag="warm", name="wps")
                    wch = min(wi * 2 * XCH // 12, 2 * XCH - 1)  # chunk index 0..7
                    cw = N // XCH
                    base = (wch % XCH) * cw
                    nc.tensor.matmul(wps[:], ones[:],
                                     xt[wch // XCH][:, base:base + NCHUNK],
                                     start=True, stop=True)
                g_ps = ps0.tile([GROUPS, 2 * XCH], F32, tag="small", name="g_ps")
                nc.tensor.matmul(g_ps[:], sel[:, 0:GROUPS], stats[0][:], start=True, stop=False)
                nc.tensor.matmul(g_ps[:], sel[:, GROUPS:2 * GROUPS], stats[1][:], start=False, stop=True)
                # per-group mean / rstd on partitions 0..7
                g_mr = cp.tile([GROUPS, 2], F32, name="g_mr", tag="g_mr")
                gtmp = cp.tile([GROUPS, 5], F32, name="gtmp", tag="gtmp")
                g_sb = cp.tile([GROUPS, 2 * XCH], F32, name="g_sb", tag="g_sb")
                nc.scalar.copy(g_sb[:], g_ps[:])
                nc.vector.tensor_reduce(gtmp[:, 3:4], g_sb[:, 0:XCH],
                                        mybir.AxisListType.X, ALU.add)
                nc.vector.tensor_reduce(gtmp[:, 4:5], g_sb[:, XCH:2 * XCH],
                                        mybir.AxisListType.X, ALU.add)
                nc.vector.tensor_scalar_mul(g_mr[:, 0:1], gtmp[:, 3:4], 1.0 / GN_COUNT)
                nc.vector.tensor_scalar_mul(gtmp[:, 0:1], gtmp[:, 4:5], 1.0 / GN_COUNT)
                nc.vector.tensor_mul(gtmp[:, 1:2], g_mr[:, 0:1], g_mr[:, 0:1])
                nc.vector.tensor_sub(gtmp[:, 2:3], gtmp[:, 0:1], gtmp[:, 1:2])
                gvar = cp.tile([GROUPS, 1], F32, name="gvar", tag="gvar")
                nc.vector.tensor_scalar_add(gvar[:], gtmp[:, 2:3], EPS)
                gstd = cp.tile([GROUPS, 1], F32, name="gstd", tag="gstd")
                nc.scalar.activation(gstd[:], gvar[:], AF.Sqrt)
                nc.vector.reciprocal(g_mr[:, 1:2], gstd[:])

                # broadcast group mean/rstd to per-channel scale/shift
                scale_t = []
                shift_t = []
                for t in range(CT):
                    mr_ps = ps0.tile([128, 2], F32, tag="small", name="mr_ps")
                    nc.tensor.matmul(mr_ps[:], selb[:, t * 128:(t + 1) * 128], g_mr[:],
                                     start=True, stop=True)
                    mr = cp.tile([128, 2], F32, name=f"mr{t}", tag=f"mr{t}")
                    nc.scalar.copy(mr[:], mr_ps[:])
                    sc = cp.tile([128, 1], F32, name=f"scale{t}", tag=f"scale{t}")
                    nc.vector.tensor_mul(sc[:], mr[:, 1:2], gamma[:, t:t + 1])
                    tmp = cp.tile([128, 1], F32, name=f"mscale{t}", tag=f"mscale{t}")
                    nc.vector.tensor_mul(tmp[:], mr[:, 0:1], sc[:])
                    sh = cp.tile([128, 1], F32, name=f"shift{t}", tag=f"shift{t}")
                    nc.vector.tensor_sub(sh[:], beta[:, t:t + 1], tmp[:])
                    scale_t.append(sc)
                    shift_t.append(sh)

                # adjusted qkv weights in fp8: wadj8[c, t, o] = wT[c, o] * scale_c
                wadj8 = cp.tile([128, CT, 3 * C], F8E4, name="wadj8", tag="wadj8")
                for t in range(CT):
                    nc.vector.tensor_scalar_mul(wadj8[:, t, :], wT[t][:], scale_t[t][:])
                # q/k bias: btot[o] = qkv_b[o] + sum_c wT[c,o]*shift_c  (o in 0..512)
                bias_ps = ps0.tile([128, 4], F32, tag="small", name="bias_ps")
                for ot in range(4):
                    for t in range(CT):
                        nc.tensor.matmul(bias_ps[:, ot:ot + 1],
                                         wT[t][:, ot * 128:(ot + 1) * 128],
                                         shift_t[t][:],
                                         start=(t == 0), stop=(t == CT - 1))
                btot = cp.tile([128, 4], F32, name="btot", tag="btot")
                nc.vector.tensor_add(btot[:], bias_ps[:], bqk[:])
                # v bias columns: bvcol[c, t] = qkv_b_v[c] + sum_i shift_i wvT[i, c]
                bv_ps = ps0.tile([128, 2], F32, tag="small", name="bv_ps")
                for t in range(CT):
                    for t2 in range(CT):
                        nc.tensor.matmul(bv_ps[:, t:t + 1],
                                         wT[t2][:, 2 * C + t * 128:2 * C + (t + 1) * 128],
                                         shift_t[t2][:],
                                         start=(t2 == 0), stop=(t2 == CT - 1))
                bvcol = cp.tile([128, 2], F32, name="bvcol", tag="bvcol")
                nc.vector.tensor_add(bvcol[:], bv_ps[:], bv[:])
                # fold v bias through proj: pbv[o] = sum_c wpT[c, o] * bvcol[c]
                pbv_ps = ps0.tile([128, 2], F32, tag="small", name="pbv_ps")
                for ot in range(CT):
                    for t in range(CT):
                        nc.tensor.matmul(pbv_ps[:, ot:ot + 1],
                                         wpt[t][:, ot * 128:(ot + 1) * 128].bitcast(F32),
                                         bvcol[:, t:t + 1],
                                         start=(t == 0), stop=(t == CT - 1))
                bp_tot = cp.tile([128, 2], F32, name="bp_tot", tag="bp_tot")
                nc.vector.tensor_add(bp_tot[:], pbv_ps[:], bp[:])

            with tc.tile_pool(name="ps1", bufs=1, space="PSUM") as ps1:
                # ---- q/k projections (DoubleRow): 2 n-chunks per PSUM tile ----
                qpack = cp.tile([128, 2, N], F8E4, name="qpack", tag="qpack")
                kpack = cp.tile([128, 2, N], F8E4, name="kpack", tag="kpack")
                # k fully first (attention needs all of k); then q in n-major
                # order so the first attention chunk unblocks after 2 evictions
                order = ([(2, 0), (3, 0), (0, 0), (1, 0)] +
                         [(ot, mcp) for mcp in range(1, NB // 2) for ot in (2, 3)] +
                         [(ot, mcp) for mcp in range(1, NB // 2) for ot in (0, 1)])
                for ev, (ot, mcp) in enumerate(order):
                    dst_t, tq = (qpack, ot) if ot < 2 else (kpack, ot - 2)
                    qk_ps = ps1.tile([128, 2 * NCHUNK], F32, tag="qk", bufs=3, name="qk_ps")
                    for half in range(2):
                        mc = 2 * mcp + half
                        nc.tensor.matmul(qk_ps[:, half * NCHUNK:(half + 1) * NCHUNK],
                                         wadj8[:, 0:2, ot * 128:(ot + 1) * 128],
                                         x8[:, 0:2, mc * NCHUNK:(mc + 1) * NCHUNK],
                                         start=True, stop=True, perf_mode=DR)
                    dst = dst_t[:, tq, 2 * mcp * NCHUNK:(2 * mcp + 2) * NCHUNK]
                    if ev % 2 == 0:
                        nc.scalar.activation(dst, qk_ps[:], AF.Identity,
                                             bias=btot[:, ot:ot + 1])
                    else:
                        nc.vector.tensor_scalar_add(dst, qk_ps[:], btot[:, ot:ot + 1])

            # vT[m, pair, parity, c] produced inside the first nb iteration so
            # its evictions pipeline with the attention start.
            vT = cp.tile([128, PAIRS, 2, C], F8E4, name="vT", tag="vT")

            with tc.tile_pool(name="ps", bufs=1, space="PSUM") as ps:
                # ---- attention ----
                # The proj/normalize epilogue of chunk nb-1 is deferred into
                # the start of chunk nb (after pair 1) so the PE never stalls
                # on the att evictions: the next chunk's S matmuls fill the
                # wait.  att evict + reciprocal stay at the end of their own
                # chunk (ring-reuse safety: readers issue before reuse).
                def emit_epilogue_pe(nsl_p, att_p, rdb_p):
                    zs = []
                    for ot in range(CT):
                        # z0 borrows the d-ring's spare slot so the s-ring
                        # keeps full depth across the chunk boundary
                        z_ps = ps.tile([128, NCHUNK], F32,
                                       tag="d" if ot == 0 else "s",
                                       bufs=2 if ot == 0 else 4, name="z_ps")
                        for t in range(CT):
                            nc.tensor.matmul(z_ps[:],
                                             wpt[t][:, ot * 128:(ot + 1) * 128],
                                             att_p[t][:],
                                             start=(t == 0), stop=(t == CT - 1))
                        zs.append(z_ps)
                    return zs

                def emit_epilogue_dve(zs, nsl_p, rdb_p):
                    for ot in range(CT):
                        y = wp.tile([128, NCHUNK], F32, tag="y", bufs=4, name="y")
                        nc.vector.tensor_mul(y[:], zs[ot][:], rdb_p[:])
                        nc.vector.scalar_tensor_tensor(
                            y[:], in0=y[:], scalar=bp_tot[:, ot:ot + 1],
                            in1=xt[ot][:, nsl_p].bitcast(F32), op0=ALU.add, op1=ALU.add)
                        nc.sync.dma_start(
                            out_d[ot * 128:(ot + 1) * 128, nsl_p], y[:])

                pending = None
                for nb in range(NB):
                    nsl = slice(nb * NCHUNK, (nb + 1) * NCHUNK)
                    out_ps = [ps.tile([128, NCHUNK], F32, tag="out", bufs=2, name=f"outp{_t}")
                              for _t in range(CT)]
                    d_ps = ps.tile([128, NCHUNK], F32, tag="d", bufs=2, name="d_ps")
                    def emit_out_d(pair, p2p):
                        first, last = (pair == 0), (pair == PAIRS - 1)
                        for t in range(CT):
                            nc.tensor.matmul(out_ps[t][:],
                                             vT[:, pair, 0:2, t * 128:(t + 1) * 128],
                                             p2p[:, 0:2, :],
                                             start=first, stop=last, perf_mode=DR)
                        nc.tensor.matmul(d_ps[:], ones8[:, 0:2, 0:128], p2p[:, 0:2, :],
                                         start=first, stop=last, perf_mode=DR)

                    # mb loop software-pipelined by one pair: pair p's S+exp
                    # issue before pair p-1's out/d matmuls, so a late exp
                    # never blocks ready S work in the PE FIFO.
                    prev_p2 = None
                    for pair in range(PAIRS):
                        if nb == 0:
                            # produce vT pair: two DoubleRow matmuls + one evict
                            vt_ps = ps.tile([128, 2 * C], F32, tag="s", bufs=4, name="vt_ps")
                            for par in range(2):
                                mt = 2 * pair + par
                                nc.tensor.matmul(vt_ps[:, par * C:(par + 1) * C],
                                                 x8[:, 0:2, mt * 128:(mt + 1) * 128],
                                                 wadj8[:, 0:2, 2 * C:3 * C],
                                                 start=True, stop=True, perf_mode=DR)
                            vdst = vT[:, pair, 0:2, :]
                            if pair % 2 == 0:
                                nc.scalar.copy(vdst, vt_ps[:])
                            else:
                                nc.vector.tensor_copy(vdst, vt_ps[:])
                        p2 = wp.tile([128, 2, NCHUNK], F8E5, tag="p", bufs=6, name="p2")
                        for par in range(2):
                            mb = 2 * pair + par
                            s_ps = ps.tile([128, NCHUNK], F32, tag="s", bufs=4, name="s_ps")
                            nc.tensor.matmul(s_ps[:],
                                             kpack[:, 0:2, mb * 128:(mb + 1) * 128],
                                             qpack[:, 0:2, nsl],
                                             start=True, stop=True, perf_mode=DR)
                            if (mb % 16) in DVE_MB:
                                nc.vector.tensor_scalar(
                                    p2[:, par, :].bitcast(I8), s_ps[:],
                                    SCH_A, SCH_B, op0=ALU.mult, op1=ALU.add)
                            else:
                                nc.scalar.activation(p2[:, par, :], s_ps[:],
                                                     AF.Exp, scale=SCALE)
                        if pair == 0 and pending is not None:
                            zs_prev = emit_epilogue_pe(*pending)
                        if pair == 2 and pending is not None:
                            emit_epilogue_dve(zs_prev, pending[0], pending[2])
                            pending = None
                        if prev_p2 is not None:
                            emit_out_d(pair - 1, prev_p2)
                        prev_p2 = p2
                    emit_out_d(PAIRS - 1, prev_p2)
                    # d_ps holds the denominator broadcast on all 128
                    # partitions (ones weights are 128 wide): approx
                    # reciprocal (18 bits, plenty) straight to SBUF
                    rdb = wp.tile([128, NCHUNK], F32, tag="rdb", bufs=2, name="rdb")
                    nc.vector.reciprocal_approx_fast(rdb[:], d_ps[:])
                    # evict attention numerator (unnormalized) to SBUF
                    att = []
                    for t in range(CT):
                        at = wp.tile([128, NCHUNK], F32R, tag="att", bufs=4, name="att")
                        if t == 0:
                            nc.scalar.copy(at[:], out_ps[t][:])
                        else:
                            nc.vector.tensor_copy(at[:], out_ps[t][:])
                        att.append(at)
                    pending = (nsl, att, rdb)
                zs_prev = emit_epilogue_pe(*pending)
                emit_epilogue_dve(zs_prev, pending[0], pending[2])
    nc.compile()
    return nc


_NC = None


def _get_nc():
    global _NC
    if _NC is None:
        _NC = _build()
    return _NC


def _prepare_in_maps(x, gn_w, gn_b, qkv_w, qkv_b, proj_w, proj_b):
    x = np.asarray(x, dtype=np.float32)
    b = x.shape[0]
    assert b == 8 and x.shape[1] == C
    xs = x.reshape(b, C, N)

    wqkvT = np.ascontiguousarray(np.asarray(qkv_w, np.float32).T)      # [C, 3C]
    wpT = np.ascontiguousarray(np.asarray(proj_w, np.float32).T)       # [C, C]
    qkv_b = np.asarray(qkv_b, np.float32)
    bqk = np.ascontiguousarray(qkv_b[:2 * C].reshape(4, 128).T)        # [128, 4]
    bv = np.ascontiguousarray(qkv_b[2 * C:].reshape(CT, 128).T)        # [128, 2]
    bp = np.ascontiguousarray(np.asarray(proj_b, np.float32).reshape(CT, 128).T)
    gamma = np.ascontiguousarray(np.asarray(gn_w, np.float32).reshape(CT, 128).T)
    beta = np.ascontiguousarray(np.asarray(gn_b, np.float32).reshape(CT, 128).T)

    # group selectors: channel c -> group c // GSIZE
    sel = np.zeros((128, 2 * GROUPS), np.float32)
    selb = np.zeros((GROUPS, C), np.float32)
    for t in range(CT):
        for p in range(128):
            g = (t * 128 + p) // GSIZE
            sel[p, t * GROUPS + g] = 1.0
            selb[g, t * 128 + p] = 1.0

    shared = {
        "wqkvT": wqkvT, "wpT": wpT, "bqk": bqk, "bv": bv, "bp": bp,
        "gamma": gamma, "beta": beta, "sel": sel, "selb": selb,
        "ones": np.ones((128, 128), np.float32),
    }
    return [dict(shared, x=np.ascontiguousarray(xs[i])) for i in range(b)]


def kernel(x, gn_w, gn_b, qkv_w, qkv_b, proj_w, proj_b):
    x = np.asarray(x, dtype=np.float32)
    in_maps = _prepare_in_maps(x, gn_w, gn_b, qkv_w, qkv_b, proj_w, proj_b)
    nc = _get_nc()
    res = run_bass_kernel_spmd(nc, in_maps, core_ids=list(range(8)))
    out = np.stack([res.results[i]["out"] for i in range(8)])
    return out.reshape(x.shape).astype(np.float32)
